# revision 12
# baseline (speedup 1.0000x reference)
"""Trainium2 Bass kernel for the attention-pooling module.

Reference math (B=32, N=2048, D=512, K=256):
    vIp   = vI @ Wi                                   [B,N,K]
    vQp   = vQ @ Wq + bq                              [B,K]
    ha    = leaky_relu(vIp + vQp[:,None,:], 0.01)     [B,N,K]
    scores= ha @ Wp[:,0] + bp                         [B,N]   (bp shift cancels in softmax)
    pi    = softmax(scores, -1)                       [B,N]
    out   = einsum("bn,bnk->bk", pi, vIp) + vQp       [B,K]

Kernel strategy (8 cores, data-parallel over B, 4 batches/core):
  - vI is host-cast to bf16 (halves HBM traffic; the pi-weighted mean is ~40x
    smaller than vQp in the output, so bf16 error in the scores/attn path is
    strongly damped).
  - vI streams in natural [N,D] layout; PE-transposes produce vIT [D,N] tiles
    feeding vIpT = Wi.T @ vI.T in [K-on-partitions, N-on-free] layout, so the
    vQp bias, Wp weighting and softmax all map onto per-partition ACT/PE ops.
  - ha = ACT Lrelu(vIpT + vQp_k) fused (per-partition bias, alpha=0.01).
  - scores = matmul(lhsT=Wp_col, rhs=ha) accumulated over the two K chunks.
  - softmax without max-subtraction (scores are provably tiny: |s| < ~2).
  - vI_attn = (e @ vI) @ Wi / Z   (exact linear refactor of pi @ vIp).
  - vQp path fully fp32 for accuracy.
"""

import os
import sys

sys.path.insert(0, "/opt/trn_rl_repo")

import numpy as np
import ml_dtypes

from concourse import bass, bacc, tile, mybir
from concourse.bass_utils import run_bass_kernel_spmd

dt = mybir.dt
F32, BF16 = dt.float32, dt.bfloat16
AF = mybir.ActivationFunctionType
ALU = mybir.AluOpType

B, N, D, K = 32, 2048, 512, 256
NCORES = 8
BLOC = B // NCORES           # 4 batches per core
NT = N // 128                # 16 N-tiles per batch
NSUP = 4                     # supertiles of 4 N-tiles (512 rows)
DC = D // 128                # 4 contraction chunks
KC = K // 128                # 2 K chunks
NEG = 0.01


DEBUG = bool(int(os.environ.get("KERNEL_DEBUG", "0")))
DBG_B = int(os.environ.get("KERNEL_DEBUG_B", "0"))


def build_nc():
    nc = bacc.Bacc("TRN2", target_bir_lowering=False, debug=False)

    vi = nc.dram_tensor("vi", [BLOC, N, D], BF16, kind="ExternalInput")
    vq = nc.dram_tensor("vq", [BLOC, D], F32, kind="ExternalInput")
    wi = nc.dram_tensor("wi", [128, DC, K], BF16, kind="ExternalInput")
    wq = nc.dram_tensor("wq", [128, DC, K], F32, kind="ExternalInput")
    bqc = nc.dram_tensor("bqc", [128, KC], F32, kind="ExternalInput")
    wpc = nc.dram_tensor("wpc", [128, KC], BF16, kind="ExternalInput")
    idb = nc.dram_tensor("idb", [128, 128], BF16, kind="ExternalInput")
    idf = nc.dram_tensor("idf", [128, 128], F32, kind="ExternalInput")
    ones = nc.dram_tensor("ones", [128, 1], F32, kind="ExternalInput")
    out = nc.dram_tensor("out", [BLOC, K], F32, kind="ExternalOutput")
    if DEBUG:
        d_vqpt = nc.dram_tensor("d_vqpt", [128, KC, BLOC], F32, kind="ExternalOutput")
        d_scrow = nc.dram_tensor("d_scrow", [1, N], F32, kind="ExternalOutput")
        d_e = nc.dram_tensor("d_e", [128, NT], BF16, kind="ExternalOutput")
        d_esum = nc.dram_tensor("d_esum", [128, 1], F32, kind="ExternalOutput")
        d_u = nc.dram_tensor("d_u", [1, D], BF16, kind="ExternalOutput")
        d_vit = nc.dram_tensor("d_vit", [128, DC, 512], BF16, kind="ExternalOutput")
        d_ha = nc.dram_tensor("d_ha", [128, 512], BF16, kind="ExternalOutput")
        d_ut = nc.dram_tensor("d_ut", [128, DC], BF16, kind="ExternalOutput")
        d_fin = nc.dram_tensor("d_fin", [1, K], F32, kind="ExternalOutput")
        d_vqpr = nc.dram_tensor("d_vqpr", [1, BLOC, K], F32, kind="ExternalOutput")

    with tile.TileContext(nc) as tc:
        with (
            tc.tile_pool(name="const", bufs=1) as cpool,
            tc.tile_pool(name="stream", bufs=6) as spool,
            tc.tile_pool(name="work", bufs=3) as wpool,
            tc.tile_pool(name="ptr", bufs=2, space=bass.MemorySpace.PSUM) as ptr,
            tc.tile_pool(name="pmm", bufs=2, space=bass.MemorySpace.PSUM) as pmm,
            tc.tile_pool(name="psm", bufs=4, space=bass.MemorySpace.PSUM) as psm,
        ):
            # ---- constants / weights ----
            wi_sb = cpool.tile([128, DC, K], BF16, tag="wi")
            wq_sb = cpool.tile([128, DC, K], F32, tag="wq")
            bq_sb = cpool.tile([128, KC], F32, tag="bq")
            wp_sb = cpool.tile([128, KC], BF16, tag="wp")
            idb_sb = cpool.tile([128, 128], BF16, tag="idb")
            idf_sb = cpool.tile([128, 128], F32, tag="idf")
            ones_sb = cpool.tile([128, 1], F32, tag="ones")
            nc.sync.dma_start(out=wi_sb[:], in_=wi[:])
            nc.sync.dma_start(out=wq_sb[:], in_=wq[:])
            nc.sync.dma_start(out=bq_sb[:], in_=bqc[:])
            nc.sync.dma_start(out=wp_sb[:], in_=wpc[:])
            nc.sync.dma_start(out=idb_sb[:], in_=idb[:])
            nc.sync.dma_start(out=idf_sb[:], in_=idf[:])
            nc.sync.dma_start(out=ones_sb[:], in_=ones[:])

            # ---- vQp (fp32, once per core, all 4 local batches) ----
            vq_sb = cpool.tile([BLOC, D], F32, tag="vqsb")
            nc.sync.dma_start(out=vq_sb[:], in_=vq[:])

            # vQ^T: [BLOC,D] -> [128, DC, BLOC]
            vqt_ps = psm.tile([128, DC, BLOC], F32, tag="small")
            for c in range(DC):
                nc.tensor.transpose(
                    vqt_ps[:, c, :],
                    vq_sb[:, c * 128 : (c + 1) * 128],
                    idf_sb[0:BLOC, 0:BLOC],
                )
            vqt_sb = cpool.tile([128, DC, BLOC], F32, tag="vqt")
            nc.vector.tensor_copy(vqt_sb[:], vqt_ps[:])

            # vQp^T[k, b] = sum_d Wq[d,k] vQ[b,d] + bq[k]   (K on partitions)
            vqpt_sb = cpool.tile([128, KC, BLOC], F32, tag="vqpt")
            for kc in range(KC):
                vqpt_ps = psm.tile([128, BLOC], F32, tag="small")
                for c in range(DC):
                    nc.tensor.matmul(
                        vqpt_ps[:],
                        wq_sb[:, c, kc * 128 : (kc + 1) * 128],
                        vqt_sb[:, c, :],
                        start=(c == 0),
                        stop=(c == DC - 1),
                    )
                nc.vector.tensor_scalar(
                    vqpt_sb[:, kc, :], vqpt_ps[:], bq_sb[:, kc : kc + 1], None, ALU.add
                )

            # row form vQp[b] = [1, K]  (transpose back; includes bq)
            vqpr_sb = cpool.tile([1, BLOC, K], F32, tag="vqpr")
            for b in range(BLOC):
                vqpr_ps = psm.tile([1, K], F32, tag="small")
                for kc in range(KC):
                    nc.tensor.transpose(
                        vqpr_ps[0:1, kc * 128 : (kc + 1) * 128],
                        vqpt_sb[:, kc, b : b + 1],
                        idf_sb[:],
                    )
                nc.vector.tensor_copy(vqpr_sb[:, b, :], vqpr_ps[:])

            out_sb = cpool.tile([1, BLOC, K], F32, tag="outb")

            # ---- per-batch pipeline ----
            for b in range(BLOC):
                # stream vI[b] in two 1 MiB halves, natural layout
                vi_t = vi[b].rearrange("(t p) d -> p t d", p=128)
                halves = []
                for h in range(2):
                    vih = spool.tile([128, NT // 2, D], BF16, tag="vih")
                    nc.sync.dma_start(out=vih[:], in_=vi_t[:, h * 8 : (h + 1) * 8, :])
                    halves.append(vih)

                scrow = wpool.tile([1, N], F32, tag="scrow")
                for s in range(NSUP):
                    h, t0 = s // 2, (s % 2) * 4
                    # transpose 4x4 128x128 tiles -> vIT [128(D), c, 512(N)]
                    vit = wpool.tile([128, DC, 512], BF16, tag="vit")
                    for c in range(DC):
                        trp = ptr.tile([128, 512], BF16, tag="trp")
                        for j in range(4):
                            nc.tensor.transpose(
                                trp[:, j * 128 : (j + 1) * 128],
                                halves[h][:, t0 + j, c * 128 : (c + 1) * 128],
                                idb_sb[:],
                            )
                        if c % 2 == 0:
                            nc.scalar.copy(vit[:, c, :], trp[:])
                        else:
                            nc.vector.tensor_copy(vit[:, c, :], trp[:])
                    if DEBUG and b == DBG_B and s == 0:
                        nc.sync.dma_start(out=d_vit[:], in_=vit[:])

                    # vIpT + fused bias/lrelu; scores via Wp-weighted column sum
                    scp = psm.tile([1, 512], F32, tag="small")
                    for kc in range(KC):
                        vp = pmm.tile([128, 512], F32, tag="vp")
                        for c in range(DC):
                            nc.tensor.matmul(
                                vp[:],
                                wi_sb[:, c, kc * 128 : (kc + 1) * 128],
                                vit[:, c, :],
                                start=(c == 0),
                                stop=(c == DC - 1),
                            )
                        ha = wpool.tile([128, 512], BF16, tag="ha")
                        nc.scalar.activation(
                            ha[:], vp[:], AF.Lrelu,
                            bias=vqpt_sb[:, kc, b : b + 1], scale=1.0, alpha=NEG,
                        )
                        nc.tensor.matmul(
                            scp[:], wp_sb[:, kc : kc + 1], ha[:],
                            start=(kc == 0), stop=(kc == KC - 1),
                        )
                        if DEBUG and b == DBG_B and s == 0 and kc == 0:
                            nc.sync.dma_start(out=d_ha[:], in_=ha[:])
                    nc.vector.tensor_copy(scrow[0:1, s * 512 : (s + 1) * 512], scp[:])

                # softmax (no max-subtraction: |scores| <~ 2)
                sct_ps = psm.tile([128, NT], F32, tag="small")
                for t in range(NT):
                    nc.tensor.transpose(
                        sct_ps[:, t : t + 1],
                        scrow[0:1, t * 128 : (t + 1) * 128],
                        idf_sb[0:1, 0:1],
                    )
                e_sb = wpool.tile([128, NT], BF16, tag="e")
                esum = wpool.tile([128, 1], F32, tag="esum")
                nc.scalar.activation(e_sb[:], sct_ps[:], AF.Exp, accum_out=esum[:])

                zps = psm.tile([1, 1], F32, tag="small")
                nc.tensor.matmul(zps[:], ones_sb[:], esum[:], start=True, stop=True)
                invz = wpool.tile([1, 1], F32, tag="invz")
                z_sb = wpool.tile([1, 1], F32, tag="zsb")
                nc.vector.tensor_copy(z_sb[:], zps[:])
                nc.vector.reciprocal(invz[:], z_sb[:])

                # u = e @ vI   [1, D]
                ups = psm.tile([1, D], F32, tag="small")
                for t in range(NT):
                    nc.tensor.matmul(
                        ups[:],
                        e_sb[:, t : t + 1],
                        halves[t // 8][:, t % 8, :],
                        start=(t == 0),
                        stop=(t == NT - 1),
                    )
                u_sb = wpool.tile([1, D], BF16, tag="usb")
                nc.vector.tensor_copy(u_sb[:], ups[:])
                if DEBUG and b == DBG_B:
                    nc.sync.dma_start(out=d_scrow[:], in_=scrow[:])
                    nc.sync.dma_start(out=d_e[:], in_=e_sb[:])
                    nc.sync.dma_start(out=d_esum[:], in_=esum[:])
                    nc.sync.dma_start(out=d_u[:], in_=u_sb[:])
                    nc.sync.dma_start(out=d_vqpt[:], in_=vqpt_sb[:])

                # bf16 psum writes must be 4B-aligned -> pad each column to 2 elems
                utp = psm.tile([128, DC, 2], BF16, tag="small")
                for c in range(DC):
                    nc.tensor.transpose(
                        utp[:, c, 0:1],
                        u_sb[0:1, c * 128 : (c + 1) * 128],
                        idb_sb[0:1, 0:1],
                    )
                ut_sb = wpool.tile([128, DC], BF16, tag="utsb")
                nc.vector.tensor_copy(ut_sb[:], utp[:, :, 0])

                # att = u @ Wi   [1, K]
                atp = psm.tile([1, K], F32, tag="small")
                for c in range(DC):
                    nc.tensor.matmul(
                        atp[:], ut_sb[:, c : c + 1], wi_sb[:, c, :],
                        start=(c == 0), stop=(c == DC - 1),
                    )
                fin = wpool.tile([1, K], F32, tag="fin")
                nc.vector.tensor_scalar(fin[:], atp[:], invz[:], None, ALU.mult)
                nc.vector.tensor_tensor(
                    out_sb[:, b, :], fin[:], vqpr_sb[:, b, :], ALU.add
                )
                if DEBUG and b == DBG_B:
                    nc.sync.dma_start(out=d_ut[:], in_=ut_sb[:])
                    nc.sync.dma_start(out=d_fin[:], in_=fin[:])
                    nc.sync.dma_start(out=d_vqpr[:], in_=vqpr_sb[:])

            nc.sync.dma_start(out=out[:, :], in_=out_sb[0:1, :, :])

    nc.compile()
    return nc


_NC = None


def _get_nc():
    global _NC
    if _NC is None:
        _NC = build_nc()
    return _NC


def kernel(vI, vQ, Wi, Wq, bq, Wp, bp, **_unused):
    vI = np.asarray(vI, dtype=np.float32)
    vQ = np.asarray(vQ, dtype=np.float32)
    Wi = np.asarray(Wi, dtype=np.float32)
    Wq = np.asarray(Wq, dtype=np.float32)
    bq = np.asarray(bq, dtype=np.float32)
    Wp = np.asarray(Wp, dtype=np.float32)
    # bp shifts every score equally -> cancels in softmax; ignored.

    bf = ml_dtypes.bfloat16
    vi_b = vI.astype(bf)                                        # [B,N,D]
    wi_h = Wi.reshape(DC, 128, K).transpose(1, 0, 2).astype(bf)  # [128,DC,K]
    wq_h = np.ascontiguousarray(Wq.reshape(DC, 128, K).transpose(1, 0, 2))
    bq_h = np.ascontiguousarray(bq.reshape(KC, 128).T)           # [128,KC]
    wp_h = np.ascontiguousarray(Wp[:, 0].reshape(KC, 128).T).astype(bf)
    idb = np.eye(128, dtype=np.float32).astype(bf)
    idf = np.eye(128, dtype=np.float32)
    ones = np.ones((128, 1), dtype=np.float32)

    in_maps = []
    for c in range(NCORES):
        in_maps.append(
            {
                "vi": np.ascontiguousarray(vi_b[c * BLOC : (c + 1) * BLOC]),
                "vq": np.ascontiguousarray(vQ[c * BLOC : (c + 1) * BLOC]),
                "wi": wi_h,
                "wq": wq_h,
                "bqc": bq_h,
                "wpc": wp_h,
                "idb": idb,
                "idf": idf,
                "ones": ones,
            }
        )

    nc = _get_nc()
    res = run_bass_kernel_spmd(
        nc, in_maps, list(range(NCORES)),
        trace=bool(int(os.environ.get("KERNEL_TRACE", "0"))),
        tmpdir=globals().get("TRACE_TMPDIR"),
    )
    kernel.last_results = res
    return np.concatenate([res.results[c]["out"] for c in range(NCORES)], axis=0)
